# revision 6
# baseline (speedup 1.0000x reference)
"""CRF forward (log-space scan) on 8 TRN2 NeuronCores — chunked bf16 scan.

Math: alpha[t,b,j] = x[b,t,j] + logsumexp_k(alpha[t-1,b,k] + T[j,k]).
In exp space with constant drift c0:  p_t = E_t * (W @ p_{t-1}),
W = exp(T), E_t = exp(x_t - c0).  Since ln p_t = ln E_t + ln(W p_{t-1})
and ln E_t = x_t - c0 is host-known, the device only emits
ln s_t, s_t = W p_{t-1} (read straight from PSUM by the scalar engine).

Time-parallel chunking: the maps p -> diag(E) W p are contractions in the
Hilbert projective metric (diag scalings are isometries; W's Birkhoff
coefficient is tanh(D/4) <= tanh(0.5) ~ 0.46 for T ~ U(0,1)), so a chunk
seeded with ones converges to the true state *direction* after ~12 steps;
the remaining per-(chunk,row) log-scale offset is recovered on the host by
matching chunk c's first output against chunk c-1's extra stitch step and
prefix-summing.  T=512 is split into K=16 chunks x L=32 steps, all chunks
advancing in lockstep in the matmul free dim: 45 macro-steps of
[128x128 block-diag W] @ [128, 512] instead of 511 serial small steps.

Per core the 128 batch rows are 4 groups x 32 classes on partitions
(block-diagonal W), free = 16 chunks x 32 rows.  bf16 weights stay
stationary in the PE array; E tiles and ln-outputs move as bf16, halving
HBM traffic.  The elementwise E-multiply is split DVE/Pool half-free each.

Chunk scheduling (micro-step i = 0..45, chunk c):
  i=0: p = 1 (memset).  i>=1: E column for chunk c is t = c*32 + i - 13.
  c=0, i=1..12:  dummy E = 1/rowsum(W) (state stays ~1, host-predicted)
  c=0, i=13:     inject exp(x_0 + orig)/s_pred  (exact t=0 start)
  c>0, i=1..12:  real warmup on t = c*32-12 .. c*32-1
  i=13..45:      outputs ln s for t = c*32 .. c*32+32 (last = stitch)
"""

import numpy as np
import ml_dtypes

import concourse.bass as bass
from concourse import bacc
import concourse.mybir as mybir
from concourse import tile
from concourse.bass_utils import run_bass_kernel_spmd

BF = ml_dtypes.bfloat16
B, T, C = 1024, 512, 32
NCORES = 8
BSH = B // NCORES          # 128 batch rows per core
NG = 4                     # row-groups stacked on partitions
P = NG * C                 # 128 partitions
K = 16                     # time chunks
L = T // K                 # 32 steps per chunk
VW = 8                     # warmup micro-steps (i=1..VW)
NSTEP = VW + L + 2         # micro-steps i=0..NSTEP-1
NI = NSTEP - 1             # E tiles (i=1..45)
NO = NSTEP - (VW + 1)      # output tiles (i=13..45) = 33
FREE = K * C               # 512 free elements (16 chunks x 32 rows)
C0 = 4.492                 # mean per-step drift of alpha

_nc_cache = None


def _build():
    global _nc_cache
    if _nc_cache is not None:
        return _nc_cache
    nc = bacc.Bacc()
    f32 = mybir.dt.float32
    bf16 = mybir.dt.bfloat16
    e_ext = nc.declare_dram_parameter("e", [NI, P, FREE], bf16, isOutput=False)
    w_ext = nc.declare_dram_parameter("w", [P, P], bf16, isOutput=False)
    o_ext = nc.declare_dram_parameter("out", [NO, P, FREE], bf16, isOutput=True)

    HF = FREE // 2
    with tile.TileContext(nc) as tc:
        with (
            tc.tile_pool(name="wpool", bufs=1) as wpool,
            tc.tile_pool(name="epool", bufs=8) as epool,
            tc.tile_pool(name="opool", bufs=6) as opool,
            tc.tile_pool(name="state", bufs=1) as spool,
            tc.tile_pool(name="psum", bufs=4, space="PSUM") as psum,
        ):
            wt = wpool.tile([P, P], bf16, name="wt")
            nc.sync.dma_start(wt[:], w_ext[:])
            # Two independent streams (chunks 0-7 / 8-15, free halves) so
            # one stream's matmul overlaps the other's DVE multiply.
            stA = [spool.tile([P, HF], bf16, tag="pA0", name="pA0"),
                   spool.tile([P, HF], bf16, tag="pA1", name="pA1")]
            stB = [spool.tile([P, HF], bf16, tag="pB0", name="pB0"),
                   spool.tile([P, HF], bf16, tag="pB1", name="pB1")]
            nc.gpsimd.memset(stA[0][:], 1.0)
            nc.gpsimd.memset(stB[0][:], 1.0)
            for i in range(1, NSTEP):
                et = epool.tile([P, FREE], bf16, tag="e")
                nc.sync.dma_start(et[:], e_ext[i - 1])
                pAp, pA = stA[(i + 1) % 2], stA[i % 2]
                pBp, pB = stB[(i + 1) % 2], stB[i % 2]
                sA = psum.tile([P, HF], f32, tag="sA")
                nc.tensor.matmul(sA[:], wt[:], pAp[:])
                # Pool/GpSimd cannot read PSUM on TRN2: both multiplies
                # live on the DVE; matmuls slot into its gaps.
                nc.vector.tensor_mul(pA[:], sA[:], et[:, 0:HF])
                sB = psum.tile([P, HF], f32, tag="sB")
                nc.tensor.matmul(sB[:], wt[:], pBp[:])
                nc.vector.tensor_mul(pB[:], sB[:], et[:, HF:])
                if i >= VW + 1:
                    ot = opool.tile([P, FREE], bf16, tag="o")
                    nc.scalar.activation(ot[:, 0:HF], sA[:],
                                         mybir.ActivationFunctionType.Ln)
                    nc.scalar.activation(ot[:, HF:], sB[:],
                                         mybir.ActivationFunctionType.Ln)
                    nc.gpsimd.dma_start(o_ext[i - VW - 1], ot[:])
    nc.compile()
    _nc_cache = nc
    return nc


def _host_consts(transition_scores):
    """Block-diag bf16 weights, dummy column, and predicted dummy state."""
    WT = np.exp(np.asarray(transition_scores, dtype=np.float64)).T  # [k, j]
    WT_bf = WT.astype(BF)
    Wblk = np.zeros((P, P), dtype=BF)
    for g in range(NG):
        Wblk[g * C:(g + 1) * C, g * C:(g + 1) * C] = WT_bf
    WT_f = WT_bf.astype(np.float32)
    r = WT_f.sum(axis=0)                               # (W @ 1)[j], f32
    Ed = (1.0 / r).astype(BF)                          # dummy E column
    # predict the chunk-0 dummy state at i=13 with device-matching rounding
    p_d = np.ones(C, dtype=BF)
    for _ in range(VW):
        s_d = p_d.astype(np.float32) @ WT_f
        p_d = (Ed.astype(np.float32) * s_d).astype(BF)
    s_pred = (p_d.astype(np.float32) @ WT_f).astype(np.float64)
    return Wblk, Ed, s_pred


def _prep_in_maps(pad_x, transition_scores, origination_scores):
    px = np.asarray(pad_x, dtype=np.float32)           # [B, T, C]
    orig = np.asarray(origination_scores, dtype=np.float64)
    Wblk, Ed, s_pred = _host_consts(transition_scores)

    X = np.exp(px - C0)                                # [B, T, C] f32
    # timestep fed at micro-step i (1..45) for chunk c: t = c*L + i - 13
    ivec = np.arange(1, NSTEP)
    tidx = (np.arange(K) * L)[None, :] + ivec[:, None] - (VW + 1)  # [45, K]
    tclip = np.clip(tidx, 0, T - 1)
    G = X[:, tclip, :]                                 # [B, 45, K, C]
    G = G.reshape(NCORES, BSH // C, C, NI, K, C)       # [core,g,rr,i,c,j]
    G = np.ascontiguousarray(G.transpose(0, 3, 1, 5, 4, 2))  # [core,i,g,j,c,rr]
    E_dev = G.reshape(NCORES, NI, P, FREE).astype(BF)

    # patches (tidx rows are i-1): chunk-0 dummy/inject; t>=T padding
    EdP = np.tile(Ed, NG)[:, None]                     # [P, 1]
    E_dev[:, 0:VW, :, 0:C] = EdP                       # c=0, i=1..12
    E_dev[:, NI - 1, :, (K - 1) * C:] = EdP            # c=K-1, i=45 (t=T)
    inj = np.exp(px[:, 0, :].astype(np.float64) + orig[None, :]) / s_pred
    inj = inj.reshape(NCORES, NG, C, C).transpose(0, 1, 3, 2)  # [core,g,j,rr]
    E_dev[:, VW, :, 0:C] = inj.reshape(NCORES, P, C).astype(BF)

    in_maps = [{"e": np.ascontiguousarray(E_dev[core]), "w": Wblk}
               for core in range(NCORES)]
    return in_maps, s_pred


def _gather(results, pad_x, origination_scores, s_pred):
    px = np.asarray(pad_x, dtype=np.float32)
    orig = np.asarray(origination_scores, dtype=np.float32)
    tvec = (C0 * np.arange(T, dtype=np.float32))[:, None, None]
    outs = []
    for core in range(NCORES):
        lo = np.asarray(results[core]["out"]).astype(np.float32)
        lo5 = lo.reshape(NO, NG, C, K, C)              # [i, g, j, c, rr]
        # stitch: chunk c's i=13 (t=cL) vs chunk c-1's i=45 (same t)
        d = (lo5[NO - 1, :, :, :-1, :] - lo5[0, :, :, 1:, :]).mean(axis=1)
        Ocorr = np.zeros((NG, K, C), dtype=np.float32)  # [g, c, rr]
        Ocorr[:, 1:, :] = np.cumsum(d, axis=1)
        A = lo5[:L].transpose(3, 0, 1, 4, 2)           # [c, ii, g, rr, j]
        A = A + Ocorr.transpose(1, 0, 2)[:, None, :, :, None]
        outs.append(A.reshape(T, BSH, C))
    alpha = np.concatenate(outs, axis=1)               # [T, B, C]
    alpha += px.transpose(1, 0, 2) - C0 + tvec
    # t=0 used the injected column: ln E = x0 + orig - ln s_pred
    alpha[0] += C0 + orig[None, :] - np.log(s_pred)[None, :].astype(np.float32)
    return alpha.astype(np.float32)


def _run(inputs, **kw):
    nc = _build()
    in_maps, s_pred = _prep_in_maps(
        inputs["pad_x"], inputs["transition_scores"],
        inputs["origination_scores"])
    res = run_bass_kernel_spmd(nc, in_maps, list(range(NCORES)), **kw)
    return res, s_pred


def _ensure_ntff_hook():
    """This image's antenv lacks axon_hooks; recreate it + register the
    ctypes NTFF hook (mirrors trn_agent_boot.trn_boot step 6)."""
    import sys
    import types
    try:
        from antenv.axon_hooks import get_axon_ntff_profile_hook  # noqa: F401
        return
    except ImportError:
        pass
    import antenv
    mod = types.ModuleType("antenv.axon_hooks")
    _h = {"hook": None}
    mod.set_axon_ntff_profile_hook = lambda h: _h.__setitem__("hook", h)
    mod.get_axon_ntff_profile_hook = lambda: _h["hook"]
    sys.modules["antenv.axon_hooks"] = mod
    antenv.axon_hooks = mod
    from trn_agent_boot.trn_boot import _ntff_profile_via_ctypes
    mod.set_axon_ntff_profile_hook(
        _ntff_profile_via_ctypes("/opt/axon/libaxon_pjrt.so"))


def run_traced(inputs, **kw):
    _ensure_ntff_hook()
    from concourse import bass_utils as bu
    bu.upload_artifacts = lambda tmpdir: "local://skipped"  # zero-egress box
    res, s_pred = _run(inputs, trace=True, **kw)
    out = _gather(res.results, inputs["pad_x"],
                  inputs["origination_scores"], s_pred)
    return out, res.exec_time_ns


def kernel(**inputs):
    res, s_pred = _run(inputs)
    return _gather(res.results, inputs["pad_x"],
                   inputs["origination_scores"], s_pred)


# revision 7
# speedup vs baseline: 1.1435x; 1.1435x over previous
"""CRF forward (log-space scan) on 8 TRN2 NeuronCores — chunked bf16 scan.

Math: alpha[t,b,j] = x[b,t,j] + logsumexp_k(alpha[t-1,b,k] + T[j,k]).
In exp space with constant drift c0:  p_t = E_t * (W @ p_{t-1}),
W = exp(T), E_t = exp(x_t - c0).  Since ln p_t = ln E_t + ln(W p_{t-1})
and ln E_t = x_t - c0 is host-known, the device only emits
ln s_t, s_t = W p_{t-1} (read straight from PSUM by the scalar engine).

Time-parallel chunking: the maps p -> diag(E) W p are contractions in the
Hilbert projective metric (diag scalings are isometries; W's Birkhoff
coefficient is tanh(D/4) <= tanh(0.5) ~ 0.46 for T ~ U(0,1)), so a chunk
seeded with ones converges to the true state *direction* after ~12 steps;
the remaining per-(chunk,row) log-scale offset is recovered on the host by
matching chunk c's first output against chunk c-1's extra stitch step and
prefix-summing.  T=512 is split into K=16 chunks x L=32 steps, all chunks
advancing in lockstep in the matmul free dim: 45 macro-steps of
[128x128 block-diag W] @ [128, 512] instead of 511 serial small steps.

Per core the 128 batch rows are 4 groups x 32 classes on partitions
(block-diagonal W), free = 16 chunks x 32 rows.  bf16 weights stay
stationary in the PE array; E tiles and ln-outputs move as bf16, halving
HBM traffic.  The elementwise E-multiply is split DVE/Pool half-free each.

Chunk scheduling (micro-step i = 0..45, chunk c):
  i=0: p = 1 (memset).  i>=1: E column for chunk c is t = c*32 + i - 13.
  c=0, i=1..12:  dummy E = 1/rowsum(W) (state stays ~1, host-predicted)
  c=0, i=13:     inject exp(x_0 + orig)/s_pred  (exact t=0 start)
  c>0, i=1..12:  real warmup on t = c*32-12 .. c*32-1
  i=13..45:      outputs ln s for t = c*32 .. c*32+32 (last = stitch)
"""

import numpy as np
import ml_dtypes

import concourse.bass as bass
from concourse import bacc
import concourse.mybir as mybir
from concourse import tile
from concourse.bass_utils import run_bass_kernel_spmd

BF = ml_dtypes.bfloat16
B, T, C = 1024, 512, 32
NCORES = 8
BSH = B // NCORES          # 128 batch rows per core
NG = 4                     # row-groups stacked on partitions
P = NG * C                 # 128 partitions
K = 16                     # time chunks
L = T // K                 # 32 steps per chunk
VW = 6                     # warmup micro-steps (i=1..VW)
NSTEP = VW + L + 2         # micro-steps i=0..NSTEP-1
NI = NSTEP - 1             # E tiles (i=1..45)
NO = NSTEP - (VW + 1)      # output tiles (i=13..45) = 33
FREE = K * C               # 512 free elements (16 chunks x 32 rows)
C0 = 4.492                 # mean per-step drift of alpha

_nc_cache = None


def _build():
    global _nc_cache
    if _nc_cache is not None:
        return _nc_cache
    nc = bacc.Bacc()
    f32 = mybir.dt.float32
    bf16 = mybir.dt.bfloat16
    e_ext = nc.declare_dram_parameter("e", [NI, P, FREE], bf16, isOutput=False)
    w_ext = nc.declare_dram_parameter("w", [P, P], bf16, isOutput=False)
    o_ext = nc.declare_dram_parameter("out", [NO, P, FREE], bf16, isOutput=True)

    HF = FREE // 2
    with tile.TileContext(nc) as tc:
        with (
            tc.tile_pool(name="wpool", bufs=1) as wpool,
            tc.tile_pool(name="epool", bufs=8) as epool,
            tc.tile_pool(name="opool", bufs=6) as opool,
            tc.tile_pool(name="state", bufs=1) as spool,
            tc.tile_pool(name="psum", bufs=4, space="PSUM") as psum,
        ):
            wt = wpool.tile([P, P], bf16, name="wt")
            nc.sync.dma_start(wt[:], w_ext[:])
            # Two independent streams (chunks 0-7 / 8-15, free halves) so
            # one stream's matmul overlaps the other's DVE multiply.
            stA = [spool.tile([P, HF], bf16, tag="pA0", name="pA0"),
                   spool.tile([P, HF], bf16, tag="pA1", name="pA1")]
            stB = [spool.tile([P, HF], bf16, tag="pB0", name="pB0"),
                   spool.tile([P, HF], bf16, tag="pB1", name="pB1")]
            nc.gpsimd.memset(stA[0][:], 1.0)
            nc.gpsimd.memset(stB[0][:], 1.0)
            for i in range(1, NSTEP):
                et = epool.tile([P, FREE], bf16, tag="e")
                nc.sync.dma_start(et[:], e_ext[i - 1])
                pAp, pA = stA[(i + 1) % 2], stA[i % 2]
                pBp, pB = stB[(i + 1) % 2], stB[i % 2]
                sA = psum.tile([P, HF], f32, tag="sA")
                nc.tensor.matmul(sA[:], wt[:], pAp[:])
                # Pool/GpSimd cannot read PSUM on TRN2: both multiplies
                # live on the DVE; matmuls slot into its gaps.
                nc.vector.tensor_mul(pA[:], sA[:], et[:, 0:HF])
                sB = psum.tile([P, HF], f32, tag="sB")
                nc.tensor.matmul(sB[:], wt[:], pBp[:])
                nc.vector.tensor_mul(pB[:], sB[:], et[:, HF:])
                if i >= VW + 1:
                    ot = opool.tile([P, FREE], bf16, tag="o")
                    nc.scalar.activation(ot[:, 0:HF], sA[:],
                                         mybir.ActivationFunctionType.Ln)
                    nc.scalar.activation(ot[:, HF:], sB[:],
                                         mybir.ActivationFunctionType.Ln)
                    nc.gpsimd.dma_start(o_ext[i - VW - 1], ot[:])
    nc.compile()
    _nc_cache = nc
    return nc


def _host_consts(transition_scores):
    """Block-diag bf16 weights, dummy column, and predicted dummy state."""
    WT = np.exp(np.asarray(transition_scores, dtype=np.float64)).T  # [k, j]
    WT_bf = WT.astype(BF)
    Wblk = np.zeros((P, P), dtype=BF)
    for g in range(NG):
        Wblk[g * C:(g + 1) * C, g * C:(g + 1) * C] = WT_bf
    WT_f = WT_bf.astype(np.float32)
    r = WT_f.sum(axis=0)                               # (W @ 1)[j], f32
    Ed = (1.0 / r).astype(BF)                          # dummy E column
    # predict the chunk-0 dummy state at i=13 with device-matching rounding
    p_d = np.ones(C, dtype=BF)
    for _ in range(VW):
        s_d = p_d.astype(np.float32) @ WT_f
        p_d = (Ed.astype(np.float32) * s_d).astype(BF)
    s_pred = (p_d.astype(np.float32) @ WT_f).astype(np.float64)
    return Wblk, Ed, s_pred


def _prep_in_maps(pad_x, transition_scores, origination_scores):
    px = np.asarray(pad_x, dtype=np.float32)           # [B, T, C]
    orig = np.asarray(origination_scores, dtype=np.float64)
    Wblk, Ed, s_pred = _host_consts(transition_scores)

    X = np.exp(px - C0)                                # [B, T, C] f32
    # timestep fed at micro-step i (1..45) for chunk c: t = c*L + i - 13
    ivec = np.arange(1, NSTEP)
    tidx = (np.arange(K) * L)[None, :] + ivec[:, None] - (VW + 1)  # [45, K]
    tclip = np.clip(tidx, 0, T - 1)
    G = X[:, tclip, :]                                 # [B, 45, K, C]
    G = G.reshape(NCORES, BSH // C, C, NI, K, C)       # [core,g,rr,i,c,j]
    G = np.ascontiguousarray(G.transpose(0, 3, 1, 5, 4, 2))  # [core,i,g,j,c,rr]
    E_dev = G.reshape(NCORES, NI, P, FREE).astype(BF)

    # patches (tidx rows are i-1): chunk-0 dummy/inject; t>=T padding
    EdP = np.tile(Ed, NG)[:, None]                     # [P, 1]
    E_dev[:, 0:VW, :, 0:C] = EdP                       # c=0, i=1..12
    E_dev[:, NI - 1, :, (K - 1) * C:] = EdP            # c=K-1, i=45 (t=T)
    inj = np.exp(px[:, 0, :].astype(np.float64) + orig[None, :]) / s_pred
    inj = inj.reshape(NCORES, NG, C, C).transpose(0, 1, 3, 2)  # [core,g,j,rr]
    E_dev[:, VW, :, 0:C] = inj.reshape(NCORES, P, C).astype(BF)

    in_maps = [{"e": np.ascontiguousarray(E_dev[core]), "w": Wblk}
               for core in range(NCORES)]
    return in_maps, s_pred


def _gather(results, pad_x, origination_scores, s_pred):
    px = np.asarray(pad_x, dtype=np.float32)
    orig = np.asarray(origination_scores, dtype=np.float32)
    tvec = (C0 * np.arange(T, dtype=np.float32))[:, None, None]
    outs = []
    for core in range(NCORES):
        lo = np.asarray(results[core]["out"]).astype(np.float32)
        lo5 = lo.reshape(NO, NG, C, K, C)              # [i, g, j, c, rr]
        # stitch: chunk c's i=13 (t=cL) vs chunk c-1's i=45 (same t)
        d = (lo5[NO - 1, :, :, :-1, :] - lo5[0, :, :, 1:, :]).mean(axis=1)
        Ocorr = np.zeros((NG, K, C), dtype=np.float32)  # [g, c, rr]
        Ocorr[:, 1:, :] = np.cumsum(d, axis=1)
        A = lo5[:L].transpose(3, 0, 1, 4, 2)           # [c, ii, g, rr, j]
        A = A + Ocorr.transpose(1, 0, 2)[:, None, :, :, None]
        outs.append(A.reshape(T, BSH, C))
    alpha = np.concatenate(outs, axis=1)               # [T, B, C]
    alpha += px.transpose(1, 0, 2) - C0 + tvec
    # t=0 used the injected column: ln E = x0 + orig - ln s_pred
    alpha[0] += C0 + orig[None, :] - np.log(s_pred)[None, :].astype(np.float32)
    return alpha.astype(np.float32)


def _run(inputs, **kw):
    nc = _build()
    in_maps, s_pred = _prep_in_maps(
        inputs["pad_x"], inputs["transition_scores"],
        inputs["origination_scores"])
    res = run_bass_kernel_spmd(nc, in_maps, list(range(NCORES)), **kw)
    return res, s_pred


def _ensure_ntff_hook():
    """This image's antenv lacks axon_hooks; recreate it + register the
    ctypes NTFF hook (mirrors trn_agent_boot.trn_boot step 6)."""
    import sys
    import types
    try:
        from antenv.axon_hooks import get_axon_ntff_profile_hook  # noqa: F401
        return
    except ImportError:
        pass
    import antenv
    mod = types.ModuleType("antenv.axon_hooks")
    _h = {"hook": None}
    mod.set_axon_ntff_profile_hook = lambda h: _h.__setitem__("hook", h)
    mod.get_axon_ntff_profile_hook = lambda: _h["hook"]
    sys.modules["antenv.axon_hooks"] = mod
    antenv.axon_hooks = mod
    from trn_agent_boot.trn_boot import _ntff_profile_via_ctypes
    mod.set_axon_ntff_profile_hook(
        _ntff_profile_via_ctypes("/opt/axon/libaxon_pjrt.so"))


def run_traced(inputs, **kw):
    _ensure_ntff_hook()
    from concourse import bass_utils as bu
    bu.upload_artifacts = lambda tmpdir: "local://skipped"  # zero-egress box
    res, s_pred = _run(inputs, trace=True, **kw)
    out = _gather(res.results, inputs["pad_x"],
                  inputs["origination_scores"], s_pred)
    return out, res.exec_time_ns


def kernel(**inputs):
    res, s_pred = _run(inputs)
    return _gather(res.results, inputs["pad_x"],
                   inputs["origination_scores"], s_pred)


# revision 8
# speedup vs baseline: 1.1680x; 1.0214x over previous
"""CRF forward (log-space scan) on 8 TRN2 NeuronCores — chunked bf16 scan.

Math: alpha[t,b,j] = x[b,t,j] + logsumexp_k(alpha[t-1,b,k] + T[j,k]).
In exp space with constant drift c0:  p_t = E_t * (W @ p_{t-1}),
W = exp(T), E_t = exp(x_t - c0).  Since ln p_t = ln E_t + ln(W p_{t-1})
and ln E_t = x_t - c0 is host-known, the device only emits
ln s_t, s_t = W p_{t-1} (read straight from PSUM by the scalar engine).

Time-parallel chunking: the maps p -> diag(E) W p are contractions in the
Hilbert projective metric (diag scalings are isometries; W's Birkhoff
coefficient is tanh(D/4) <= tanh(0.5) ~ 0.46 for T ~ U(0,1)), so a chunk
seeded with ones converges to the true state *direction* after ~12 steps;
the remaining per-(chunk,row) log-scale offset is recovered on the host by
matching chunk c's first output against chunk c-1's extra stitch step and
prefix-summing.  T=512 is split into K=16 chunks x L=32 steps, all chunks
advancing in lockstep in the matmul free dim: 45 macro-steps of
[128x128 block-diag W] @ [128, 512] instead of 511 serial small steps.

Per core the 128 batch rows are 4 groups x 32 classes on partitions
(block-diagonal W), free = 16 chunks x 32 rows.  bf16 weights stay
stationary in the PE array; E tiles and ln-outputs move as bf16, halving
HBM traffic.  The elementwise E-multiply is split DVE/Pool half-free each.

Chunk scheduling (micro-step i = 0..45, chunk c):
  i=0: p = 1 (memset).  i>=1: E column for chunk c is t = c*32 + i - 13.
  c=0, i=1..12:  dummy E = 1/rowsum(W) (state stays ~1, host-predicted)
  c=0, i=13:     inject exp(x_0 + orig)/s_pred  (exact t=0 start)
  c>0, i=1..12:  real warmup on t = c*32-12 .. c*32-1
  i=13..45:      outputs ln s for t = c*32 .. c*32+32 (last = stitch)
"""

import numpy as np
import ml_dtypes

import concourse.bass as bass
from concourse import bacc
import concourse.mybir as mybir
from concourse import tile
from concourse.bass_utils import run_bass_kernel_spmd

BF = ml_dtypes.bfloat16
B, T, C = 1024, 512, 32
NCORES = 8
BSH = B // NCORES          # 128 batch rows per core
NG = 4                     # row-groups stacked on partitions
P = NG * C                 # 128 partitions
K = 16                     # time chunks
L = T // K                 # 32 steps per chunk
VW = 4                     # warmup micro-steps (i=1..VW)
NSTEP = VW + L + 2         # micro-steps i=0..NSTEP-1
NI = NSTEP - 1             # E tiles (i=1..45)
NO = NSTEP - (VW + 1)      # output tiles (i=13..45) = 33
FREE = K * C               # 512 free elements (16 chunks x 32 rows)
C0 = 4.492                 # mean per-step drift of alpha

_nc_cache = None


def _build():
    global _nc_cache
    if _nc_cache is not None:
        return _nc_cache
    nc = bacc.Bacc()
    f32 = mybir.dt.float32
    bf16 = mybir.dt.bfloat16
    e_ext = nc.declare_dram_parameter("e", [NI, P, FREE], bf16, isOutput=False)
    w_ext = nc.declare_dram_parameter("w", [P, P], bf16, isOutput=False)
    o_ext = nc.declare_dram_parameter("out", [NO, P, FREE], bf16, isOutput=True)

    HF = FREE // 2
    with tile.TileContext(nc) as tc:
        with (
            tc.tile_pool(name="wpool", bufs=1) as wpool,
            tc.tile_pool(name="epool", bufs=8) as epool,
            tc.tile_pool(name="opool", bufs=6) as opool,
            tc.tile_pool(name="state", bufs=1) as spool,
            tc.tile_pool(name="psum", bufs=4, space="PSUM") as psum,
        ):
            wt = wpool.tile([P, P], bf16, name="wt")
            nc.sync.dma_start(wt[:], w_ext[:])
            # Two independent streams (chunks 0-7 / 8-15, free halves) so
            # one stream's matmul overlaps the other's DVE multiply.
            stA = [spool.tile([P, HF], bf16, tag="pA0", name="pA0"),
                   spool.tile([P, HF], bf16, tag="pA1", name="pA1")]
            stB = [spool.tile([P, HF], bf16, tag="pB0", name="pB0"),
                   spool.tile([P, HF], bf16, tag="pB1", name="pB1")]
            nc.gpsimd.memset(stA[0][:], 1.0)
            nc.gpsimd.memset(stB[0][:], 1.0)
            for i in range(1, NSTEP):
                et = epool.tile([P, FREE], bf16, tag="e")
                nc.sync.dma_start(et[:], e_ext[i - 1])
                pAp, pA = stA[(i + 1) % 2], stA[i % 2]
                pBp, pB = stB[(i + 1) % 2], stB[i % 2]
                sA = psum.tile([P, HF], f32, tag="sA")
                nc.tensor.matmul(sA[:], wt[:], pAp[:])
                # Pool/GpSimd cannot read PSUM on TRN2: both multiplies
                # live on the DVE; matmuls slot into its gaps.
                nc.vector.tensor_mul(pA[:], sA[:], et[:, 0:HF])
                sB = psum.tile([P, HF], f32, tag="sB")
                nc.tensor.matmul(sB[:], wt[:], pBp[:])
                nc.vector.tensor_mul(pB[:], sB[:], et[:, HF:])
                if i >= VW + 1:
                    ot = opool.tile([P, FREE], bf16, tag="o")
                    nc.scalar.activation(ot[:, 0:HF], sA[:],
                                         mybir.ActivationFunctionType.Ln)
                    nc.scalar.activation(ot[:, HF:], sB[:],
                                         mybir.ActivationFunctionType.Ln)
                    nc.gpsimd.dma_start(o_ext[i - VW - 1], ot[:])
    nc.compile()
    _nc_cache = nc
    return nc


def _host_consts(transition_scores):
    """Block-diag bf16 weights, dummy column, and predicted dummy state."""
    WT = np.exp(np.asarray(transition_scores, dtype=np.float64)).T  # [k, j]
    WT_bf = WT.astype(BF)
    Wblk = np.zeros((P, P), dtype=BF)
    for g in range(NG):
        Wblk[g * C:(g + 1) * C, g * C:(g + 1) * C] = WT_bf
    WT_f = WT_bf.astype(np.float32)
    r = WT_f.sum(axis=0)                               # (W @ 1)[j], f32
    Ed = (1.0 / r).astype(BF)                          # dummy E column
    # predict the chunk-0 dummy state at i=13 with device-matching rounding
    p_d = np.ones(C, dtype=BF)
    for _ in range(VW):
        s_d = p_d.astype(np.float32) @ WT_f
        p_d = (Ed.astype(np.float32) * s_d).astype(BF)
    s_pred = (p_d.astype(np.float32) @ WT_f).astype(np.float64)
    return Wblk, Ed, s_pred


def _prep_in_maps(pad_x, transition_scores, origination_scores):
    px = np.asarray(pad_x, dtype=np.float32)           # [B, T, C]
    orig = np.asarray(origination_scores, dtype=np.float64)
    Wblk, Ed, s_pred = _host_consts(transition_scores)

    X = np.exp(px - C0)                                # [B, T, C] f32
    # timestep fed at micro-step i (1..45) for chunk c: t = c*L + i - 13
    ivec = np.arange(1, NSTEP)
    tidx = (np.arange(K) * L)[None, :] + ivec[:, None] - (VW + 1)  # [45, K]
    tclip = np.clip(tidx, 0, T - 1)
    G = X[:, tclip, :]                                 # [B, 45, K, C]
    G = G.reshape(NCORES, BSH // C, C, NI, K, C)       # [core,g,rr,i,c,j]
    G = np.ascontiguousarray(G.transpose(0, 3, 1, 5, 4, 2))  # [core,i,g,j,c,rr]
    E_dev = G.reshape(NCORES, NI, P, FREE).astype(BF)

    # patches (tidx rows are i-1): chunk-0 dummy/inject; t>=T padding
    EdP = np.tile(Ed, NG)[:, None]                     # [P, 1]
    E_dev[:, 0:VW, :, 0:C] = EdP                       # c=0, i=1..12
    E_dev[:, NI - 1, :, (K - 1) * C:] = EdP            # c=K-1, i=45 (t=T)
    inj = np.exp(px[:, 0, :].astype(np.float64) + orig[None, :]) / s_pred
    inj = inj.reshape(NCORES, NG, C, C).transpose(0, 1, 3, 2)  # [core,g,j,rr]
    E_dev[:, VW, :, 0:C] = inj.reshape(NCORES, P, C).astype(BF)

    in_maps = [{"e": np.ascontiguousarray(E_dev[core]), "w": Wblk}
               for core in range(NCORES)]
    return in_maps, s_pred


def _gather(results, pad_x, origination_scores, s_pred):
    px = np.asarray(pad_x, dtype=np.float32)
    orig = np.asarray(origination_scores, dtype=np.float32)
    tvec = (C0 * np.arange(T, dtype=np.float32))[:, None, None]
    outs = []
    for core in range(NCORES):
        lo = np.asarray(results[core]["out"]).astype(np.float32)
        lo5 = lo.reshape(NO, NG, C, K, C)              # [i, g, j, c, rr]
        # stitch: chunk c's i=13 (t=cL) vs chunk c-1's i=45 (same t)
        d = (lo5[NO - 1, :, :, :-1, :] - lo5[0, :, :, 1:, :]).mean(axis=1)
        Ocorr = np.zeros((NG, K, C), dtype=np.float32)  # [g, c, rr]
        Ocorr[:, 1:, :] = np.cumsum(d, axis=1)
        A = lo5[:L].transpose(3, 0, 1, 4, 2)           # [c, ii, g, rr, j]
        A = A + Ocorr.transpose(1, 0, 2)[:, None, :, :, None]
        outs.append(A.reshape(T, BSH, C))
    alpha = np.concatenate(outs, axis=1)               # [T, B, C]
    alpha += px.transpose(1, 0, 2) - C0 + tvec
    # t=0 used the injected column: ln E = x0 + orig - ln s_pred
    alpha[0] += C0 + orig[None, :] - np.log(s_pred)[None, :].astype(np.float32)
    return alpha.astype(np.float32)


def _run(inputs, **kw):
    nc = _build()
    in_maps, s_pred = _prep_in_maps(
        inputs["pad_x"], inputs["transition_scores"],
        inputs["origination_scores"])
    res = run_bass_kernel_spmd(nc, in_maps, list(range(NCORES)), **kw)
    return res, s_pred


def _ensure_ntff_hook():
    """This image's antenv lacks axon_hooks; recreate it + register the
    ctypes NTFF hook (mirrors trn_agent_boot.trn_boot step 6)."""
    import sys
    import types
    try:
        from antenv.axon_hooks import get_axon_ntff_profile_hook  # noqa: F401
        return
    except ImportError:
        pass
    import antenv
    mod = types.ModuleType("antenv.axon_hooks")
    _h = {"hook": None}
    mod.set_axon_ntff_profile_hook = lambda h: _h.__setitem__("hook", h)
    mod.get_axon_ntff_profile_hook = lambda: _h["hook"]
    sys.modules["antenv.axon_hooks"] = mod
    antenv.axon_hooks = mod
    from trn_agent_boot.trn_boot import _ntff_profile_via_ctypes
    mod.set_axon_ntff_profile_hook(
        _ntff_profile_via_ctypes("/opt/axon/libaxon_pjrt.so"))


def run_traced(inputs, **kw):
    _ensure_ntff_hook()
    from concourse import bass_utils as bu
    bu.upload_artifacts = lambda tmpdir: "local://skipped"  # zero-egress box
    res, s_pred = _run(inputs, trace=True, **kw)
    out = _gather(res.results, inputs["pad_x"],
                  inputs["origination_scores"], s_pred)
    return out, res.exec_time_ns


def kernel(**inputs):
    res, s_pred = _run(inputs)
    return _gather(res.results, inputs["pad_x"],
                   inputs["origination_scores"], s_pred)
